# revision 15
# baseline (speedup 1.0000x reference)
"""Trainium2 Bass kernel for nn_DigitCap (capsule DigitCaps layer).

Math: the reference's routing loop is degenerate — softmax over a size-1
axis is exactly 1.0, so c_ij == 1 on every iteration and the output only
depends on s[b,l,o] = sum_{p,n} W[0,p,l,o,n] * x[b,n,p], followed by the
squash nonlinearity (norm taken over the L axis, faithful to the source):

    m2[b,o]    = sum_l s[b,l,o]^2
    out[b,l,o] = s[b,l,o] * sqrt(m2[b,o]) / (1 + m2[b,o])

This collapses to one (256 x 9216) @ (9216 x 160) matmul plus a tiny
elementwise epilogue.

Sharding over 8 NeuronCores: split the contraction dim K = N*P = 9216 by
the N axis (1152 rows of K per core; W is split, not replicated, and each
core reads only its slice of x).  Each core computes a partial
s_partial (256, 160); an on-device collective combines the partials:

  - mode "ar": AllReduce; every core then applies squash to the full
    (256,160) and writes the full output (host takes core 0's copy).
  - mode "ag": AllGather; every core sums the 8 partials locally, then
    as in "ar".
  - mode "rs": ReduceScatter over the batch axis; core i squashes and
    writes batch rows [32i, 32i+32) (host concatenates).
  - mode "a2a": AllToAll of batch shards + local tree-sum; like "rs".

Layout notes:
  - per-core input "xt" is x[:, i, :].T  (K=1152, B=256) so the matmul's
    stationary operand (lhsT) is a direct slice.
  - per-core input "w" is W[0, :, :, :, i] with free dim ordered f=o*10+l
    so the squash's l-reduction is over the innermost axis.
  - host converts the (256, 160) gathered result back to (256, 10, 16).
"""

import numpy as np

B, N, P, L, O = 256, 8, 1152, 10, 16
NCORES = 8
KC = P // 128          # 9 k-chunks of 128 per core
BB = B // NCORES       # 32 batch rows per core in the scatter modes
LO = L * O             # 160

MODE = "bp"

GP = 4                 # col-tiled k-chunks per PE pass in "bp" mode
NPASS = N * P // 128 // GP   # 18 passes over the full K for one core

_cache = {}


def _emit_squash(nc, mybir, post, s, nrows, idx):
    """Emit squash for an SBUF tile s of shape [nrows, LO]; returns v tile."""
    f32 = mybir.dt.float32
    sq = post.tile([nrows, LO], f32, name=f"sq{idx}")
    m2 = post.tile([nrows, O], f32, name=f"m2{idx}")
    rt = post.tile([nrows, O], f32, name=f"rt{idx}")
    dn = post.tile([nrows, O], f32, name=f"dn{idx}")
    tf = post.tile([nrows, O], f32, name=f"tf{idx}")
    vv = post.tile([nrows, LO], f32, name=f"vv{idx}")
    nc.vector.tensor_mul(sq[:], s[:], s[:])
    nc.vector.reduce_sum(
        m2[:], sq[:].rearrange("b (o l) -> b o l", l=L),
        axis=mybir.AxisListType.X)
    nc.scalar.activation(rt[:], m2[:], mybir.ActivationFunctionType.Sqrt)
    nc.vector.tensor_scalar_add(dn[:], m2[:], 1.0)
    nc.vector.reciprocal(dn[:], dn[:])
    nc.vector.tensor_mul(tf[:], rt[:], dn[:])
    nc.vector.tensor_mul(
        vv[:].rearrange("b (o l) -> b o l", l=L),
        s[:].rearrange("b (o l) -> b o l", l=L),
        tf[:][:, :, None].broadcast_to([nrows, O, L]))
    return vv


def _build(mode=MODE):
    if mode in _cache:
        return _cache[mode]

    import concourse.bacc as bacc
    import concourse.mybir as mybir
    import concourse.tile as tile

    f32 = mybir.dt.float32
    nc = bacc.Bacc("TRN2", target_bir_lowering=False, debug=False,
                   num_devices=NCORES)
    if mode == "bp":
        return _build_bp(nc, mybir)
    if mode == "bp2":
        return _build_bp2(nc, mybir)
    xt_d = nc.dram_tensor("xt", [P, B], f32, kind="ExternalInput").ap()
    w_d = nc.dram_tensor("w", [P, LO], f32, kind="ExternalInput").ap()
    out_rows = BB if mode in ("rs", "a2a") else B
    out_d = nc.dram_tensor("out", [out_rows, LO], f32,
                           kind="ExternalOutput").ap()

    with tile.TileContext(nc) as tc:
        with (
            tc.tile_pool(name="io", bufs=3) as io_pool,
            tc.tile_pool(name="ps", bufs=1, space="PSUM") as ps_pool,
            tc.tile_pool(name="dram", bufs=1, space="DRAM") as dram_pool,
            tc.tile_pool(name="post", bufs=1) as post,
        ):
            xt_v = xt_d.rearrange("(c p) b -> c p b", p=128)
            w_v = w_d.rearrange("(c p) f -> c p f", p=128)
            ps0 = ps_pool.tile([128, LO], f32, name="ps0")
            ps1 = ps_pool.tile([128, LO], f32, name="ps1")
            for c in range(KC):
                xt_t = io_pool.tile([128, B], f32, tag="xt", name=f"xt{c}")
                w_t = io_pool.tile([128, LO], f32, tag="w", name=f"w{c}")
                nc.sync.dma_start(xt_t[:], xt_v[c])
                nc.sync.dma_start(w_t[:], w_v[c])
                nc.tensor.matmul(ps0[:], xt_t[:, 0:128], w_t[:],
                                 start=(c == 0), stop=(c == KC - 1))
                nc.tensor.matmul(ps1[:], xt_t[:, 128:256], w_t[:],
                                 start=(c == 0), stop=(c == KC - 1))

            partial = dram_pool.tile([B, LO], f32, name="partial")
            s0 = post.tile([128, LO], f32, name="s0")
            s1 = post.tile([128, LO], f32, name="s1")
            nc.vector.tensor_copy(s0[:], ps0[:])
            nc.vector.tensor_copy(s1[:], ps1[:])
            nc.sync.dma_start(partial[0:128, :], s0[:])
            nc.sync.dma_start(partial[128:256, :], s1[:])

            rg = [list(range(NCORES))]
            if mode == "ar":
                red = dram_pool.tile([B, LO], f32, name="red",
                                     addr_space="Shared")
                nc.gpsimd.collective_compute(
                    "AllReduce", mybir.AluOpType.add, replica_groups=rg,
                    ins=[partial.opt()], outs=[red.opt()])
                for h in range(2):
                    sh = post.tile([128, LO], f32, name=f"sh{h}")
                    nc.sync.dma_start(sh[:], red[128 * h:128 * (h + 1), :])
                    vv = _emit_squash(nc, mybir, post, sh, 128, h)
                    nc.sync.dma_start(out_d[128 * h:128 * (h + 1), :], vv[:])
            elif mode == "ag":
                red = dram_pool.tile([NCORES * B, LO], f32, name="red",
                                     addr_space="Shared")
                nc.gpsimd.collective_compute(
                    "AllGather", mybir.AluOpType.bypass, replica_groups=rg,
                    ins=[partial.opt()], outs=[red.opt()])
                red_v = red.rearrange("(r b) f -> b r f", b=B)
                for h in range(2):
                    r8 = post.tile([128, NCORES, LO], f32, name=f"r8{h}")
                    nc.sync.dma_start(r8[:], red_v[128 * h:128 * (h + 1)])
                    sh = post.tile([128, LO], f32, name=f"sh{h}")
                    nc.vector.reduce_sum(
                        sh[:], r8[:].rearrange("b r f -> b f r"),
                        axis=mybir.AxisListType.X)
                    vv = _emit_squash(nc, mybir, post, sh, 128, h)
                    nc.sync.dma_start(out_d[128 * h:128 * (h + 1), :], vv[:])
            elif mode == "rs":
                red = dram_pool.tile([BB, LO], f32, name="red")
                nc.gpsimd.collective_compute(
                    "ReduceScatter", mybir.AluOpType.add, replica_groups=rg,
                    ins=[partial.opt()], outs=[red.opt()])
                s = post.tile([BB, LO], f32, name="s")
                nc.sync.dma_start(s[:], red[:])
                vv = _emit_squash(nc, mybir, post, s, BB, 0)
                nc.sync.dma_start(out_d[:], vv[:])
            else:  # a2a
                red = dram_pool.tile([B, LO], f32, name="red")
                nc.gpsimd.collective_compute(
                    "AllToAll", mybir.AluOpType.bypass, replica_groups=rg,
                    ins=[partial.opt()], outs=[red.opt()])
                r8 = post.tile([BB, NCORES, LO], f32, name="r8")
                nc.sync.dma_start(r8[:], red.rearrange("(r b) f -> b r f",
                                                       b=BB))
                s = post.tile([BB, LO], f32, name="s")
                nc.vector.reduce_sum(
                    s[:], r8[:].rearrange("b r f -> b f r"),
                    axis=mybir.AxisListType.X)
                vv = _emit_squash(nc, mybir, post, s, BB, 0)
                nc.sync.dma_start(out_d[:], vv[:])

    nc.compile()
    _cache[mode] = nc
    return nc


def _build_bp(nc, mybir):
    """Batch-parallel: W replicated, batch sharded 8 x 32, no collective.

    PE efficiency at M=32 is recovered with 4x column tiling: each PE pass
    runs 4 k-chunks concurrently in the four 32-column groups of the array,
    accumulating into four disjoint 32-partition strips of one PSUM tile.
    The four strips are partial K-sums, added together on DVE at the end.
    DMA is split across both HWDGE queues (sync + scalar)."""
    import concourse.tile as tile

    f32 = mybir.dt.float32
    K = N * P
    xt_d = nc.dram_tensor("xt", [K, BB], f32, kind="ExternalInput").ap()
    w_d = nc.dram_tensor("w", [K, LO], f32, kind="ExternalInput").ap()
    sel_d = nc.dram_tensor("sel", [128, BB], f32, kind="ExternalInput").ap()
    out_d = nc.dram_tensor("out", [BB, LO], f32, kind="ExternalOutput").ap()

    with tile.TileContext(nc) as tc:
        with (
            tc.tile_pool(name="io", bufs=3) as io_pool,
            tc.tile_pool(name="ps", bufs=1, space="PSUM") as ps_pool,
            tc.tile_pool(name="post", bufs=1) as post,
        ):
            xt_v = xt_d.rearrange("(g j p) m -> g p j m", j=GP, p=128)
            w_v = w_d.rearrange("(g j p) f -> g p j f", j=GP, p=128)
            sel_t = post.tile([128, BB], f32, name="sel_t")
            nc.scalar.dma_start(sel_t[:], sel_d[:])
            ps = ps_pool.tile([128, LO], f32, name="ps")
            for g in range(NPASS):
                xt_t = io_pool.tile([128, GP, BB], f32, tag="xt",
                                    name=f"xt{g}")
                w_t = io_pool.tile([128, GP, LO], f32, tag="w", name=f"w{g}")
                dma_eng = nc.sync if g % 2 == 0 else nc.scalar
                xt_eng = nc.scalar if g % 2 == 0 else nc.sync
                xt_eng.dma_start(xt_t[:], xt_v[g])
                dma_eng.dma_start(w_t[:], w_v[g])
                for j in range(GP):
                    nc.tensor.matmul(
                        ps[32 * j:32 * (j + 1), :], xt_t[:, j, :],
                        w_t[:, j, :], start=(g == 0), stop=(g == NPASS - 1),
                        tile_position=(0, 32 * j))

            # sum the four 32-partition strips: s = sel.T @ sp on the PE
            # (DVE cannot add across base partitions; walrus rejects it).
            sp = post.tile([128, LO], f32, name="sp")
            nc.vector.tensor_copy(sp[:], ps[:])
            ps2 = ps_pool.tile([BB, LO], f32, name="ps2")
            nc.tensor.matmul(ps2[:], sel_t[:], sp[:], start=True, stop=True)
            s = post.tile([BB, LO], f32, name="s")
            nc.vector.tensor_copy(s[:], ps2[:])
            vv = _emit_squash(nc, mybir, post, s, BB, 0)
            nc.sync.dma_start(out_d[:], vv[:])

    nc.compile()
    _cache["bp"] = nc
    return nc


def _build_bp2(nc, mybir):
    """Like bp, but inputs are host-packed so each PE pass's W/xt tile is a
    contiguous DRAM block (per-partition runs of 1280B/512B instead of
    640B/128B), and every W pass-load is split across both HWDGE queues."""
    import concourse.tile as tile

    f32 = mybir.dt.float32
    xt_d = nc.dram_tensor("xt", [NPASS * 128, GP * BB], f32,
                          kind="ExternalInput").ap()
    w_d = nc.dram_tensor("w", [NPASS * 128, GP * LO], f32,
                         kind="ExternalInput").ap()
    sel_d = nc.dram_tensor("sel", [128, BB], f32, kind="ExternalInput").ap()
    out_d = nc.dram_tensor("out", [BB, LO], f32, kind="ExternalOutput").ap()

    with tile.TileContext(nc) as tc:
        with (
            tc.tile_pool(name="io", bufs=4) as io_pool,
            tc.tile_pool(name="ps", bufs=1, space="PSUM") as ps_pool,
            tc.tile_pool(name="post", bufs=1) as post,
        ):
            xt_v = xt_d.rearrange("(g p) m -> g p m", p=128)
            w_v = w_d.rearrange("(g p) f -> g p f", p=128)
            sel_t = post.tile([128, BB], f32, name="sel_t")
            nc.gpsimd.dma_start(sel_t[:], sel_d[:])
            ps = ps_pool.tile([128, LO], f32, name="ps")
            half = GP // 2 * LO
            for g in range(NPASS):
                xt_t = io_pool.tile([128, GP * BB], f32, tag="xt",
                                    name=f"xt{g}")
                w_t = io_pool.tile([128, GP * LO], f32, tag="w", name=f"w{g}")
                e0, e1 = (nc.sync, nc.scalar) if g % 2 == 0 else \
                         (nc.scalar, nc.sync)
                e0.dma_start(w_t[:, 0:half], w_v[g][:, 0:half])
                e1.dma_start(w_t[:, half:], w_v[g][:, half:])
                nc.gpsimd.dma_start(xt_t[:], xt_v[g])
                for j in range(GP):
                    nc.tensor.matmul(
                        ps[32 * j:32 * (j + 1), :],
                        xt_t[:, BB * j:BB * (j + 1)],
                        w_t[:, LO * j:LO * (j + 1)],
                        start=(g == 0), stop=(g == NPASS - 1),
                        tile_position=(0, 32 * j))

            sp = post.tile([128, LO], f32, name="sp")
            nc.vector.tensor_copy(sp[:], ps[:])
            ps2 = ps_pool.tile([BB, LO], f32, name="ps2")
            nc.tensor.matmul(ps2[:], sel_t[:], sp[:], start=True, stop=True)
            s = post.tile([BB, LO], f32, name="s")
            nc.vector.tensor_copy(s[:], ps2[:])
            vv = _emit_squash(nc, mybir, post, s, BB, 0)
            nc.sync.dma_start(out_d[:], vv[:])

    nc.compile()
    _cache["bp2"] = nc
    return nc


def _prep_inputs(x, W, mode=MODE):
    x = np.asarray(x, dtype=np.float32)
    W = np.asarray(W, dtype=np.float32)
    if mode == "bp2":
        # pack so each pass's tile is one contiguous DRAM block:
        # packed[g, p, j*D+d] = flat[128*(GP*g+j)+p, d]
        wf = np.ascontiguousarray(
            W[0].transpose(3, 0, 2, 1).reshape(N * P, LO))
        w2 = np.ascontiguousarray(
            wf.reshape(NPASS, GP, 128, LO).transpose(0, 2, 1, 3)
            .reshape(NPASS * 128, GP * LO))
        sel = np.zeros((128, BB), np.float32)
        sel[np.arange(128), np.arange(128) % BB] = 1.0
        in_maps = []
        for i in range(NCORES):
            xt = x[BB * i:BB * (i + 1)].reshape(BB, N * P).T  # (9216, 32)
            x2 = np.ascontiguousarray(
                xt.reshape(NPASS, GP, 128, BB).transpose(0, 2, 1, 3)
                .reshape(NPASS * 128, GP * BB))
            in_maps.append({"xt": x2, "w": w2, "sel": sel})
        return in_maps
    if mode == "bp":
        # xt = per-core batch-slice of x, flattened (b, n*p) and transposed;
        # w = full W with rows k=(n,p), cols f=o*10+l — identical per core.
        wf = np.ascontiguousarray(
            W[0].transpose(3, 0, 2, 1).reshape(N * P, LO))    # (9216, 160)
        sel = np.zeros((128, BB), np.float32)
        sel[np.arange(128), np.arange(128) % BB] = 1.0
        in_maps = []
        for i in range(NCORES):
            xs = x[BB * i:BB * (i + 1)].reshape(BB, N * P)
            in_maps.append({"xt": np.ascontiguousarray(xs.T), "w": wf,
                            "sel": sel})
        return in_maps
    in_maps = []
    for i in range(NCORES):
        xt = np.ascontiguousarray(x[:, i, :].T)               # (1152, 256)
        w = np.ascontiguousarray(
            W[0, :, :, :, i].transpose(0, 2, 1).reshape(P, LO))  # (1152, 160)
        in_maps.append({"xt": xt, "w": w})
    return in_maps


def _postprocess(results, mode=MODE):
    if mode in ("rs", "a2a", "bp", "bp2"):
        full = np.concatenate([results[i]["out"] for i in range(NCORES)],
                              axis=0)
    else:
        full = results[0]["out"]
    return np.ascontiguousarray(
        full.reshape(B, O, L).transpose(0, 2, 1))             # (256, 10, 16)


def kernel(x, W):
    from concourse.bass_utils import run_bass_kernel_spmd

    nc = _build(MODE)
    res = run_bass_kernel_spmd(nc, _prep_inputs(x, W, MODE),
                               core_ids=list(range(NCORES)))
    return _postprocess(res.results)
